# revision 14
# baseline (speedup 1.0000x reference)
"""
CIN (Compressed Interaction Network) kernel for Trainium2, 8 NeuronCores.

Problem (hardcoded):
  x: [4096, 32, 64] fp32; w0: [128, 1024]; b0: [128]; w1: [128, 2048]; b1: [128]
  out: [4096, 192] = concat(relu(y0)[:, 64:], relu(y1)).sum(d)

Sharding: data parallel over batch, 512 samples/core, tokens t=(b,d), T=32768.

Per-core pipeline (pair = 1024 tokens = 16 samples, 32 pairs):
  - Layer 0 uses the symmetry of x (x) x: only the 528 pairs (h<=f) are kept,
    host-packed into 5 groups of 128 rows with folded weights
    w0sym[o,(h,f)] = w0[o,hf]+w0[o,fh].  A/B side tiles (pure gathers of x
    rows) stream from HBM; z0 = A*B mostly on VectorE (bf16 2x), some GpSimd.
  - Layer 1 hidden broadcast: 4 concurrent 32-row one-hot matmuls
    (tile_position rows 0/32/64/96) write 4 PSUM banks; row p = hid[4g+p//32].
    Evacuation per group mode: 'A' = ScalarE copy to SBUF bf16 then VectorE
    2x multiply; 'V' = VectorE multiplies directly from PSUM (1x).
  - W matmuls accumulate over groups in PSUM, g outer (production order).
  - d-sum reduces on VectorE in bf16 (2x), delayed one pair to avoid stalls.
"""

import sys

import numpy as np
import ml_dtypes

sys.path.insert(0, "/opt/trn_rl_repo")

B_FULL = 4096
N_CORES = 8
BS = B_FULL // N_CORES  # 512
F = 32
D = 64
T = BS * D  # 32768
PAIR = 1024  # tokens per pair (16 samples)
O = 128
H1 = 64
G0 = 5  # sym-packed layer-0 groups (528 pairs + pad -> 640 rows)
G1 = 16

# layer-1 group production/consumption order: batches of 4 quarters
PROD = [0, 8, 1, 9, 2, 10, 3, 11, 4, 12, 5, 13, 6, 14, 7, 15]
# evac mode per group: 'A' ScalarE evac + DVE 2x mul; 'V' DVE direct 1x mul
MODE = {g: ("V" if g in (4, 12, 5) else "A") for g in range(G1)}
# z0 groups multiplied on gpsimd instead of DVE
Z0_GPSIMD = (3, 4)

_CACHE = {}


def _sym_pairs():
    pairs = [(h, f) for h in range(F) for f in range(h, F)]
    pairs += [(0, 0)] * (G0 * 128 - len(pairs))
    return pairs


def _quarter(g):
    # g even/odd selects natural/duplicate copy; g//8 selects hid half
    return 2 * (g % 2) + g // 8


def _build_nc(BS=BS, PAIR=PAIR):
    import concourse.bass as bass
    import concourse.tile as tile
    from concourse import bacc, mybir

    T = BS * D
    NPAIR = T // PAIR
    SPP = PAIR // D  # samples per pair

    bf16 = mybir.dt.bfloat16
    f32 = mybir.dt.float32
    Relu = mybir.ActivationFunctionType.Relu
    Copy = mybir.ActivationFunctionType.Copy
    X = mybir.AxisListType.X
    ADD = mybir.AluOpType.add

    nc = bacc.Bacc(None, target_bir_lowering=False)

    xr2 = nc.dram_tensor("xr2", [128, T], bf16, kind="ExternalInput")
    abd = nc.dram_tensor("abd", [G0, 2, 128, T], bf16, kind="ExternalInput")
    w0t = nc.dram_tensor("w0t", [G0 * 128, O], bf16, kind="ExternalInput")
    w1t = nc.dram_tensor("w1t", [G1 * 128, O], bf16, kind="ExternalInput")
    sel4 = nc.dram_tensor("sel4", [128, G1, 128], bf16, kind="ExternalInput")
    b0 = nc.dram_tensor("b0", [O, 1], f32, kind="ExternalInput")
    b1 = nc.dram_tensor("b1", [O, 1], f32, kind="ExternalInput")
    out0 = nc.dram_tensor("out0", [O - H1, BS], bf16, kind="ExternalOutput")
    out1 = nc.dram_tensor("out1", [O, BS], bf16, kind="ExternalOutput")

    with tile.TileContext(nc) as tc:
        with (
            tc.tile_pool(name="singles", bufs=1) as singles,
            tc.tile_pool(name="xrp", bufs=3) as xrp,
            tc.tile_pool(name="ab", bufs=7) as abp,
            tc.tile_pool(name="z0", bufs=7) as z0p,
            tc.tile_pool(name="bcsb", bufs=16) as bcsbp,
            tc.tile_pool(name="z1", bufs=18) as z1p,
            tc.tile_pool(name="ysb", bufs=5) as ysbp,
            tc.tile_pool(name="hdup", bufs=2) as hdupp,
            tc.tile_pool(name="hp", bufs=4, space="PSUM") as hpp,
            tc.tile_pool(name="py0", bufs=1, space="PSUM") as py0p,
            tc.tile_pool(name="py1", bufs=1, space="PSUM") as py1p,
        ):
            w0s = singles.tile([128, G0, O], bf16)
            w1s = singles.tile([128, G1, O], bf16)
            s4s = singles.tile([128, G1, 128], bf16)
            b0s = singles.tile([O, 1], f32)
            b1s = singles.tile([O, 1], f32)
            oacc0 = singles.tile([O - H1, BS], bf16)
            oacc1 = singles.tile([O, BS], bf16)

            nc.gpsimd.dma_start(out=w0s[:], in_=w0t.rearrange("(g k) m -> k g m", k=128))
            nc.gpsimd.dma_start(out=w1s[:], in_=w1t.rearrange("(g k) m -> k g m", k=128))
            nc.gpsimd.dma_start(out=s4s[:], in_=sel4[:])
            nc.gpsimd.dma_start(out=b0s[:], in_=b0[:])
            nc.gpsimd.dma_start(out=b1s[:], in_=b1[:])

            pending_red = []

            for P in range(NPAIR):
                sl = slice(P * PAIR, (P + 1) * PAIR)
                xr = xrp.tile([128, PAIR], bf16)
                nc.gpsimd.dma_start(out=xr[:], in_=xr2[:, sl])

                # ---- layer 0: sym-packed A/B from HBM ----
                z0s = []
                for g in range(G0):
                    ab = abp.tile([128, 2, PAIR], bf16)
                    nc.gpsimd.dma_start(
                        out=ab[:], in_=abd[g][:, :, sl].rearrange("two k t -> k two t")
                    )
                    z = z0p.tile([128, PAIR], bf16)
                    eng = nc.gpsimd if g in Z0_GPSIMD else nc.vector
                    eng.tensor_mul(z[:], ab[:, 0, :], ab[:, 1, :])
                    z0s.append(z)

                # reduces of previous pair (inputs long since ready)
                for red in pending_red:
                    red()
                pending_red = []

                py0 = py0p.tile([O, PAIR], f32)
                for g in range(G0):
                    for s in range(PAIR // 512):
                        cs = slice(s * 512, (s + 1) * 512)
                        nc.tensor.matmul(
                            py0[:, cs], w0s[:, g, :], z0s[g][:, cs],
                            start=(g == 0), stop=(g == G0 - 1),
                        )
                y0sb = ysbp.tile([128, PAIR], bf16)
                nc.scalar.activation(y0sb[0:64, :], py0[0:64, :], Relu, bias=b0s[0:64, :])
                nc.scalar.activation(y0sb[64:128, :], py0[64:128, :], Relu, bias=b0s[64:128, :])

                hdup = hdupp.tile([128, PAIR], bf16)
                nc.gpsimd.dma_start(out=hdup[64:128, :], in_=y0sb[0:64, :])

                # ---- layer 1 broadcast: quartets of 32-row one-hot matmuls ----
                z1s = {}
                for b4 in range(0, G1, 4):
                    quart = PROD[b4 : b4 + 4]
                    hps = {}
                    stage = {}
                    for g in quart:
                        z1s[g] = z1p.tile([128, PAIR], bf16, name="z1")
                        if MODE[g] == "A":
                            stage[g] = bcsbp.tile([128, PAIR], bf16, name="bcsb")
                        elif MODE[g] == "D":
                            stage[g] = bcsbp.tile([128, PAIR], bf16, name="bcsb")
                            nc.sync.dma_start(
                                out=stage[g][:],
                                in_=y0sb[4 * g : 4 * g + 4, :]
                                .unsqueeze(1)
                                .broadcast_to([4, 32, PAIR]),
                            )
                    for s in range(PAIR // 512):
                        cs = slice(s * 512, (s + 1) * 512)
                        for g in quart:
                            if MODE[g] == "D":
                                continue
                            q = _quarter(g)
                            src = y0sb if q < 2 else hdup
                            hp = hpp.tile([128, 512], f32, name="hp")
                            nc.tensor.matmul(
                                hp[:],
                                s4s[32 * q : 32 * (q + 1), g, :],
                                src[32 * q : 32 * (q + 1), cs],
                                start=True, stop=True,
                                tile_position=(32 * q, 0),
                            )
                            hps[(g, s)] = hp
                        for g in quart:
                            if MODE[g] == "D":
                                continue
                            hp = hps[(g, s)]
                            if MODE[g] == "A":
                                nc.scalar.activation(stage[g][:, cs], hp[:], Copy)
                            else:
                                nc.vector.tensor_mul(
                                    z1s[g][:, cs], hp[:], xr[:, cs]
                                )
                    for g in quart:
                        if MODE[g] in ("A", "D"):
                            nc.vector.tensor_mul(z1s[g][:], stage[g][:], xr[:])

                py1 = py1p.tile([O, PAIR], f32)
                for i, g in enumerate(PROD):
                    for s in range(PAIR // 512):
                        cs = slice(s * 512, (s + 1) * 512)
                        nc.tensor.matmul(
                            py1[:, cs], w1s[:, g, :], z1s[g][:, cs],
                            start=(i == 0), stop=(i == G1 - 1),
                        )
                y1sb = ysbp.tile([128, PAIR], bf16)
                nc.scalar.activation(y1sb[:], py1[:], Relu, bias=b1s[:])

                def make_red(P=P, y0sb=y0sb, y1sb=y1sb):
                    def red():
                        with nc.allow_low_precision(reason="64-term d-sum in bf16"):
                            nc.vector.tensor_reduce(
                                oacc0[:, P * SPP : (P + 1) * SPP],
                                y0sb[H1:O, :].rearrange("p (b d) -> p b d", d=D),
                                axis=X, op=ADD,
                            )
                            nc.vector.tensor_reduce(
                                oacc1[:, P * SPP : (P + 1) * SPP],
                                y1sb[:].rearrange("p (b d) -> p b d", d=D),
                                axis=X, op=ADD,
                            )
                    return red

                pending_red.append(make_red())

            for red in pending_red:
                red()
            nc.gpsimd.dma_start(out=out0[:], in_=oacc0[:])
            nc.gpsimd.dma_start(out=out1[:], in_=oacc1[:])

    nc.finalize()
    return nc


def _get_nc():
    if "nc" not in _CACHE:
        _CACHE["nc"] = _build_nc()
    return _CACHE["nc"]


def make_sel4():
    """One-hot mats, row-major: group g output row p reads hid[4g + p//32],
    staged in quarter q = 2*(g%2) + g//8 as local row j = 4*(g%8) + p//32."""
    sel = np.zeros((128, G1, 128), np.float32)
    for g in range(G1):
        q = _quarter(g)
        for p in range(128):
            j = 4 * (g % 8) + p // 32
            sel[32 * q + j, g, p] = 1.0
    return sel.astype(ml_dtypes.bfloat16)


def kernel(cin_inputs, w0, b0, w1, b1, _trace=False):
    from concourse.bass_utils import run_bass_kernel_spmd

    x = np.asarray(cin_inputs, dtype=np.float32)
    assert x.shape == (B_FULL, F, D)
    bf = ml_dtypes.bfloat16
    w0f = np.asarray(w0, dtype=np.float32)
    w1f = np.asarray(w1, dtype=np.float32)

    # per-core token layout: xt[f, t], t = b*64 + d
    xt_all = np.ascontiguousarray(
        x.reshape(N_CORES, BS, F, D).transpose(0, 2, 1, 3)
    ).reshape(N_CORES, F, T)

    # L1 moving x side (row-major bc layout): xr2[p] = x[p % 32]
    xr2_all = np.tile(xt_all, (1, 4, 1)).astype(bf)  # [N, 128, T]

    # L0 sym-packed A/B sides
    pairs = _sym_pairs()
    h_idx = np.array([h for h, f in pairs])
    f_idx = np.array([f for h, f in pairs])
    ab_all = np.stack(
        [xt_all[:, h_idx, :], xt_all[:, f_idx, :]], axis=1
    )  # [N, 2, 640, T]
    ab_all = np.ascontiguousarray(
        ab_all.reshape(N_CORES, 2, G0, 128, T).transpose(0, 2, 1, 3, 4)
    ).astype(bf)  # [N, G0, 2, 128, T]

    # folded sym layer-0 weights
    w0sym = np.zeros((O, G0 * 128), np.float32)
    w0m = w0f.reshape(O, F, F)
    for s, (h, f) in enumerate(pairs[:528]):
        w0sym[:, s] = w0m[:, h, f] + (w0m[:, f, h] if h != f else 0.0)
    w0t = np.ascontiguousarray(w0sym.T).astype(bf)  # [640, O]

    # L1 weights, row-major: row 128g+p -> w1[o, 32h+f], h=4g+p//32, f=p%32
    p = np.arange(128)
    g = np.arange(G1)
    hh = 4 * g[:, None] + (p // 32)[None, :]
    ff = np.broadcast_to((p % 32)[None, :], (G1, 128))
    cidx = (32 * hh + ff).reshape(-1)
    w1t = np.ascontiguousarray(w1f[:, cidx].T).astype(bf)  # [2048, O]

    b0c = np.asarray(b0, dtype=np.float32).reshape(O, 1).copy()
    b1c = np.asarray(b1, dtype=np.float32).reshape(O, 1).copy()
    s4 = make_sel4()

    nc = _get_nc()
    in_maps = []
    for i in range(N_CORES):
        in_maps.append(
            {
                "xr2": xr2_all[i],
                "abd": ab_all[i],
                "w0t": w0t, "w1t": w1t,
                "sel4": s4, "b0": b0c, "b1": b1c,
            }
        )
    res = run_bass_kernel_spmd(nc, in_maps, core_ids=list(range(N_CORES)), trace=_trace)
    outs = []
    for r in res.results:
        o = np.concatenate(
            [np.asarray(r["out0"], dtype=np.float32),
             np.asarray(r["out1"], dtype=np.float32)], axis=0
        ).T
        outs.append(o)
    full = np.concatenate(outs, axis=0).astype(np.float32)
    if _trace:
        return full, res
    return full
